# revision 7
# baseline (speedup 1.0000x reference)
"""
BasicCrossAttention Trainium2 kernel (8 NeuronCores, SPMD head-parallel).

Sharding: 16 heads split across 8 cores (2 heads/core).  Each core computes
Q/K/V projections for its 2 heads (column-sharded Wq/Wk/Wv), per-head QK
LayerNorm, full cross-attention over B*H_local, and a row-sharded partial of
the output projection.  The host sums the 8 partial outputs and adds bias.

Device math is bf16 matmuls with fp32 PSUM accumulation.

v1 changes over the original baseline (396us):
  - x1/x2 are pre-transposed on the HOST -> xT tiles load as plain strided
    DMAs on the hw DGE (no serialized Sync-queue DMA-xbar transposes, no
    256B descriptor shatter).
  - softmax normalizer reciprocal uses the single-instruction
    reciprocal_approx_fast DVE op (~640ns vs 3.2us bit-exact iterative
    divide) -- removes the 16 head-of-line Vector-queue stalls that idled
    the whole machine ~4-5us each.
  - output partials are written bf16 (host sums in f32).
  - emission schedule weaves projection row-groups, attention chunks, and
    output-projection chunks so the ACT engine (softmax exp, the scarcest
    resource after PE) starts ~15us in and never drains until the tail.
"""

import os
import sys

for _p in ("/root/.axon_site", "/root/.axon_site/_ro/trn_rl_repo",
           "/root/.axon_site/_ro/pypackages", "/opt/trn_rl_repo"):
    if os.path.isdir(_p) and _p not in sys.path:
        sys.path.append(_p)

import numpy as np
import ml_dtypes
from contextlib import ExitStack

B = 2
N = 2048          # query rows (x1)
M = 2048          # key rows (x2)
DM = 1024         # d_model
H = 16            # total heads
HD = 64           # head dim
NCORES = 8
HL = H // NCORES  # heads per core = 2
LOC = HL * HD     # local feature width = 128
SCALE = 8.0 / HD  # mup scale
EPS = 1e-5

_COMPILED = None          # cached Bass program
LAST_RESULT = None        # BassKernelResults of last run (for profiling)


def _emit(ctx, tc, aps):
    import concourse.bass as bass
    from concourse import mybir
    from concourse.masks import make_identity

    nc = tc.nc
    f32 = mybir.dt.float32
    bf16 = mybir.dt.bfloat16
    AF = mybir.ActivationFunctionType
    OP = mybir.AluOpType

    x1T, x2T, wqT, wkT, wvT, wp, ln_g, ln_b, out = (
        aps["x1T"], aps["x2T"], aps["wqT"], aps["wkT"], aps["wvT"],
        aps["wp"], aps["ln_g"], aps["ln_b"], aps["out"])

    const = ctx.enter_context(tc.tile_pool(name="const", bufs=1))
    xT_pool = ctx.enter_context(tc.tile_pool(name="xTp", bufs=2))
    nat_pool = ctx.enter_context(tc.tile_pool(name="natp", bufs=10))
    stat_pool = ctx.enter_context(tc.tile_pool(name="statp", bufs=3))
    big_pool = ctx.enter_context(tc.tile_pool(name="bigp", bufs=2))
    pT_pool = ctx.enter_context(tc.tile_pool(name="pTp", bufs=3))
    out_pool = ctx.enter_context(tc.tile_pool(name="outp", bufs=2))
    bc_pool = ctx.enter_context(tc.tile_pool(name="bcp", bufs=2))
    ps128 = ctx.enter_context(tc.tile_pool(name="ps128", bufs=2, space="PSUM"))
    psbig = ctx.enter_context(tc.tile_pool(name="psbig", bufs=2, space="PSUM"))

    # ---------------- constants / weights ----------------
    ident = const.tile([128, 128], bf16)
    make_identity(nc, ident)

    # wkv layout: [in 128, fc 8, k(128) | v(128)]
    wkv_sb = const.tile([128, 8, 2 * LOC], bf16)
    nc.gpsimd.dma_start(wkv_sb[:, :, 0:LOC],
                        wkT.rearrange("(a p) o -> p a o", p=128))
    nc.gpsimd.dma_start(wkv_sb[:, :, LOC:2 * LOC],
                        wvT.rearrange("(a p) o -> p a o", p=128))
    wq_sb = const.tile([128, 8, LOC], bf16)
    nc.gpsimd.dma_start(wq_sb, wqT.rearrange("(a p) o -> p a o", p=128))
    wp_sb = const.tile([128, DM], bf16)
    nc.gpsimd.dma_start(wp_sb, wp)

    # ln params replicated per local feature: partition p <- param[p % 64]
    g_col = const.tile([128, 1], f32)
    nc.gpsimd.dma_start(g_col, bass.AP(tensor=ln_g.tensor, offset=ln_g.offset,
                                       ap=[[0, HL], [1, HD]]))
    b_col = const.tile([128, 1], f32)
    nc.gpsimd.dma_start(b_col, bass.AP(tensor=ln_b.tensor, offset=ln_b.offset,
                                       ap=[[0, HL], [1, HD]]))
    gq_col = const.tile([128, 1], f32)
    nc.vector.tensor_scalar_mul(gq_col, g_col, SCALE)
    bq_col = const.tile([128, 1], f32)
    nc.vector.tensor_scalar_mul(bq_col, b_col, SCALE)

    # Mean-center the K and Q weight head-blocks so projections are zero-mean
    # per head (LN mean handled in the weights; only E[x^2] needed per tile).
    for w_sb, nblk in ((wkv_sb, HL), (wq_sb, HL)):
        for fc in range(8):
            for h in range(nblk):
                blk = w_sb[:, fc, h * HD:(h + 1) * HD]
                m = stat_pool.tile([128, 1], f32, tag="wm")
                nc.vector.reduce_sum(m, blk, axis=mybir.AxisListType.X)
                nc.vector.tensor_scalar_mul(m, m, 1.0 / HD)
                nc.vector.tensor_scalar(blk, blk, m, None, op0=OP.subtract)

    # persistent per-batch tiles (bufs=2 -> both batches in flight)
    kT = [None, None]
    qT = [None, None]
    Vt = [None, None]
    hoT = [None, None]

    # ---------------- phase generators ----------------
    def proj(b, is_q, rg):
        """Project K|V (from x2T) or Q (from x1T) for one 1024-row group.

        Emits the xT load as one big strided DMA, then the 8 m-tile matmul
        groups + LN stats, then the batched rstd, then per m-tile the
        normalize + transpose + scale-out.  Yields once after each m-tile's
        kT/qT columns are fully written (so attention over that tile may be
        emitted by the driver)."""
        if is_q and rg == 0:
            qT[b] = big_pool.tile([128, N], bf16, tag="qT", name=f"qT{b}")
        if not is_q and rg == 0:
            kT[b] = big_pool.tile([128, M], bf16, tag="kT", name=f"kT{b}")
            Vt[b] = big_pool.tile([128, 16, 2 * (HD + 1)], bf16, tag="V",
                                  name=f"V{b}")
        src = x1T if is_q else x2T
        w_sb = wq_sb if is_q else wkv_sb
        nout = LOC if is_q else 2 * LOC
        dst = qT[b] if is_q else kT[b]
        gc, bc = (gq_col, bq_col) if is_q else (g_col, b_col)

        xT = xT_pool.tile([128, 8, 1024], bf16, tag="xT",
                          name=f"xT{b}{int(is_q)}{rg}")
        nc.sync.dma_start(
            xT, src[b, :, rg * 1024:(rg + 1) * 1024].rearrange(
                "(a p) n -> p a n", p=128))

        s2g = stat_pool.tile([128, 8, HL], f32, tag="s2g",
                             name=f"s2g{b}{int(is_q)}{rg}")
        raws = []
        for mi in range(8):
            mt = rg * 8 + mi  # global 128-row tile index
            rs = slice(mi * 128, (mi + 1) * 128)
            ps = ps128.tile([128, nout], f32, tag="ps128",
                            name=f"ps{b}{int(is_q)}{mt}")
            for fc in range(8):
                nc.tensor.matmul(ps, lhsT=xT[:, fc, rs],
                                 rhs=w_sb[:, fc, :],
                                 start=(fc == 0), stop=(fc == 7))
            raw = nat_pool.tile([128, LOC], bf16, tag="raw", bufs=10,
                                name=f"raw{b}{int(is_q)}{mt}")
            nc.vector.tensor_copy(raw, ps[:, 0:LOC])
            raws.append(raw)
            # E[x^2] per head for LN (weights are centered)
            sq = nat_pool.tile([128, LOC], f32, tag="sq", bufs=2)
            nc.vector.tensor_mul(sq, raw, raw)
            nc.vector.reduce_sum(s2g[:, mi, :],
                                 sq.rearrange("p (h d) -> p h d", h=HL),
                                 axis=mybir.AxisListType.X)
            if not is_q:
                vt = Vt[b][:, mt, :]
                nc.gpsimd.memset(vt[:, HD::HD + 1], 1.0)
                vt3 = bass.AP(tensor=vt.tensor, offset=vt.offset,
                              ap=[vt.ap[0], [HD + 1, HL], [1, HD]])
                nc.vector.tensor_copy(
                    vt3, ps[:, LOC:2 * LOC].rearrange(
                        "p (h x) -> p h x", h=HL))
        # per-row-group rstd on DVE: rsqrt(var+eps) via linear seed
        # + 3 Newton steps (keeps ACT exclusively on softmax exp)
        rstdg = stat_pool.tile([128, 8, HL], f32, tag="rstdg")
        y = rstdg.rearrange("p a b -> p (a b)")
        var = stat_pool.tile([128, 8 * HL], f32, tag="lnvar")
        tnr = stat_pool.tile([128, 8 * HL], f32, tag="lntnr")
        nc.vector.tensor_scalar(var, s2g.rearrange("p a b -> p (a b)"),
                                1.0 / HD, EPS, op0=OP.mult, op1=OP.add)
        nc.vector.tensor_scalar(y, var, -0.315, 1.43,
                                op0=OP.mult, op1=OP.add)
        for _ in range(3):
            nc.vector.tensor_mul(tnr, y, y)
            nc.vector.tensor_mul(tnr, tnr, var)
            nc.vector.tensor_scalar(tnr, tnr, -0.5, 1.5,
                                    op0=OP.mult, op1=OP.add)
            nc.vector.tensor_mul(y, y, tnr)
        for mi in range(8):
            mt = rg * 8 + mi
            nrm = nat_pool.tile([128, LOC], bf16, tag="nrm", bufs=3)
            for h in range(HL):
                hs = slice(h * HD, (h + 1) * HD)
                nc.vector.tensor_scalar(
                    nrm[:, hs], raws[mi][:, hs],
                    rstdg[:, mi, h:h + 1], None, op0=OP.mult)
            tps = ps128.tile([128, 128], bf16, tag="ps128",
                             name=f"tps{b}{int(is_q)}{mt}")
            nc.tensor.transpose(tps, nrm, ident)
            nc.vector.tensor_scalar(
                dst[:, mt * 128:(mt + 1) * 128], tps, gc, bc,
                op0=OP.mult, op1=OP.add)
            yield  # m-tile `mt` of dst is complete

    def attn_mc(b, nc4):
        """S^T -> exp -> (V|1)^T @ P^T for one 512-query chunk; one yield
        per key m-tile (16 total).  Must be pulled only after the
        corresponding kT/Vt m-tile has been emitted."""
        ns = slice(nc4 * 512, (nc4 + 1) * 512)
        av = psbig.tile([128, 1024], f32, tag="av", bufs=1,
                        name=f"av{b}{nc4}")
        avs[(b, nc4)] = av
        for mc in range(16):
            mcs = slice(mc * 128, (mc + 1) * 128)
            st = psbig.tile([128, 1024], f32, tag="st",
                            name=f"st{b}{nc4}{mc}")
            for h in range(HL):
                nc.tensor.matmul(st[:, h * 512:(h + 1) * 512],
                                 lhsT=kT[b][h * HD:(h + 1) * HD, mcs],
                                 rhs=qT[b][h * HD:(h + 1) * HD, ns],
                                 start=True, stop=True)
            pT = pT_pool.tile([128, 1024], bf16, tag="pT")
            nc.scalar.activation(pT, st, AF.Exp)
            for h in range(HL):
                nc.tensor.matmul(
                    av[0:HD + 1, h * 512:(h + 1) * 512],
                    lhsT=Vt[b][:, mc, h * (HD + 1):(h + 1) * (HD + 1)],
                    rhs=pT[:, h * 512:(h + 1) * 512],
                    start=(mc == 0), stop=(mc == 15),
                    skip_group_check=True)
            yield

    avs = {}

    def attn_norm(b, nc4):
        """Drain the accumulator, normalize by the softmax sum, write hoT."""
        if hoT[b] is None:
            hoT[b] = big_pool.tile([128, N], bf16, tag="hoT", name=f"hoT{b}")
        ns = slice(nc4 * 512, (nc4 + 1) * 512)
        av = avs.pop((b, nc4))
        # drain the accumulator to SBUF fast (frees the PSUM bank so the
        # next chunk's AV matmuls never stall on the normalize chain)
        av_sb = bc_pool.tile([HD + 1, 1024], f32, tag="avsb")
        nc.any.tensor_copy(av_sb, av[0:HD + 1, :])
        # 1/Z via exp(-ln Z) on ACT: both functions share one table set
        # (natural_log_exp_and_others), and ACT row ops don't head-of-line
        # block the DVE the way the 3.2us iterative reciprocal did.
        lnz = bc_pool.tile([1, 1024], f32, tag="lnz")
        nc.scalar.activation(lnz, av_sb[HD:HD + 1, :], AF.Ln)
        recip = bc_pool.tile([1, 1024], f32, tag="recip")
        nc.scalar.activation(recip, lnz, AF.Exp, scale=-1.0)
        for h in range(HL):
            hs = slice(h * HD, (h + 1) * HD)
            sl = slice(h * 512, (h + 1) * 512)
            bcast = bc_pool.tile([HD, 512], f32, tag="bcast")
            nc.gpsimd.partition_broadcast(bcast, recip[:, sl])
            nc.gpsimd.tensor_mul(hoT[b][hs, ns], av_sb[0:HD, sl], bcast)

    def outp(b, nc4):
        """Output projection partials for the 4 query m-tiles of chunk nc4.
        Must be emitted after attn_norm(b, nc4)."""
        for nt in range(4 * nc4, 4 * nc4 + 4):
            osb = out_pool.tile([128, DM], bf16, tag="osb")
            for oc in range(4):
                fps = ps128.tile([128, 256], f32, tag="ps128",
                                 name=f"fps{b}{nt}{oc}")
                nc.tensor.matmul(fps,
                                 lhsT=hoT[b][:, nt * 128:(nt + 1) * 128],
                                 rhs=wp_sb[:, oc * 256:(oc + 1) * 256],
                                 start=True, stop=True)
                nc.any.tensor_copy(osb[:, oc * 256:(oc + 1) * 256], fps)
                if oc == 3:
                    nc.sync.dma_start(
                        out[b, nt * 128:(nt + 1) * 128, :], osb)
                yield

    # ---------------- drivers ----------------
    def run(g):
        for _ in g:
            pass

    def weave(producer, *consumers):
        """Drain `producer`; after each of its yields (one per completed
        m-tile), pull `n` steps from each (gen, n) consumer."""
        for _ in producer:
            for g, n in consumers:
                for _ in range(n):
                    next(g, None)

    A = {}
    O = {}

    # --- batch 0 production, woven with batch-0 attention ---
    run(proj(0, True, 0))                              # Q(b0) tiles 0-7
    A[0, 0] = attn_mc(0, 0)
    weave(proj(0, False, 0), (A[0, 0], 1))             # K|V rg0 ; attn mc 0-7
    weave(proj(0, False, 1), (A[0, 0], 1))             # K|V rg1 ; attn mc 8-15
    attn_norm(0, 0)
    A[0, 1] = attn_mc(0, 1)
    weave(proj(0, True, 1), (A[0, 1], 2))              # Q rg1 ; attn chunk 1
    attn_norm(0, 1)
    # --- batch 1 production, woven with attn(0) tail + outp(0) ---
    A[0, 2] = attn_mc(0, 2)
    O[0, 0] = outp(0, 0)
    O[0, 1] = outp(0, 1)
    weave(proj(1, False, 0), (A[0, 2], 1), (O[0, 0], 2))
    weave(proj(1, False, 1), (A[0, 2], 1), (O[0, 1], 2))
    attn_norm(0, 2)
    A[0, 3] = attn_mc(0, 3)
    O[0, 2] = outp(0, 2)
    weave(proj(1, True, 0), (A[0, 3], 2), (O[0, 2], 2))
    attn_norm(0, 3)
    A[1, 0] = attn_mc(1, 0)
    O[0, 3] = outp(0, 3)
    weave(proj(1, True, 1), (A[1, 0], 2), (O[0, 3], 2))
    attn_norm(1, 0)
    # --- batch 1 attention + output drain ---
    for c in range(1, 4):
        A[1, c] = attn_mc(1, c)
        O[1, c - 1] = outp(1, c - 1)
        weave(A[1, c], (O[1, c - 1], 1))
        attn_norm(1, c)
    run(outp(1, 3))
    # safety: drain any generator with leftover steps (counts should be exact)
    for g in list(A.values()) + list(O.values()):
        run(g)


def _build():
    global _COMPILED
    if _COMPILED is not None:
        return _COMPILED
    import concourse.tile as tile
    from concourse import bacc, mybir

    nc = bacc.Bacc("TRN2", target_bir_lowering=False, debug=False,
                   enable_asserts=False)
    bf16 = mybir.dt.bfloat16
    f32 = mybir.dt.float32
    aps = {
        "x1T": nc.dram_tensor("x1T", [B, DM, N], bf16, kind="ExternalInput").ap(),
        "x2T": nc.dram_tensor("x2T", [B, DM, M], bf16, kind="ExternalInput").ap(),
        "wqT": nc.dram_tensor("wqT", [DM, LOC], bf16, kind="ExternalInput").ap(),
        "wkT": nc.dram_tensor("wkT", [DM, LOC], bf16, kind="ExternalInput").ap(),
        "wvT": nc.dram_tensor("wvT", [DM, LOC], bf16, kind="ExternalInput").ap(),
        "wp": nc.dram_tensor("wp", [LOC, DM], bf16, kind="ExternalInput").ap(),
        "ln_g": nc.dram_tensor("ln_g", [HD], f32, kind="ExternalInput").ap(),
        "ln_b": nc.dram_tensor("ln_b", [HD], f32, kind="ExternalInput").ap(),
        "out": nc.dram_tensor("out", [B, N, DM], bf16, kind="ExternalOutput").ap(),
    }
    with tile.TileContext(nc) as tc, ExitStack() as ctx:
        _emit(ctx, tc, aps)
    nc.compile()
    _COMPILED = nc
    return nc


def kernel(x1, x2, Wq, Wk, Wv, Wp, bp, ln_g, ln_b):
    global LAST_RESULT
    from concourse.bass_utils import run_bass_kernel_spmd

    nc = _build()
    bf = ml_dtypes.bfloat16
    x1T = np.ascontiguousarray(
        np.asarray(x1, dtype=np.float32).transpose(0, 2, 1)).astype(bf)
    x2T = np.ascontiguousarray(
        np.asarray(x2, dtype=np.float32).transpose(0, 2, 1)).astype(bf)
    Wq = np.asarray(Wq, dtype=np.float32)
    Wk = np.asarray(Wk, dtype=np.float32)
    Wv = np.asarray(Wv, dtype=np.float32)
    Wp = np.asarray(Wp, dtype=np.float32)
    ln_g32 = np.ascontiguousarray(np.asarray(ln_g, dtype=np.float32))
    ln_b32 = np.ascontiguousarray(np.asarray(ln_b, dtype=np.float32))

    in_maps = []
    for c in range(NCORES):
        hs = slice(c * LOC, (c + 1) * LOC)
        in_maps.append({
            "x1T": x1T,
            "x2T": x2T,
            "wqT": np.ascontiguousarray(Wq[hs, :].T).astype(bf),
            "wkT": np.ascontiguousarray(Wk[hs, :].T).astype(bf),
            "wvT": np.ascontiguousarray(Wv[hs, :].T).astype(bf),
            "wp": np.ascontiguousarray(Wp[:, hs].T).astype(bf),
            "ln_g": ln_g32,
            "ln_b": ln_b32,
        })

    res = run_bass_kernel_spmd(nc, in_maps, core_ids=list(range(NCORES)))
    LAST_RESULT = res
    acc = np.zeros((B, N, DM), dtype=np.float32)
    for r in res.results:
        acc += np.asarray(r["out"], dtype=np.float32)
    acc += np.asarray(bp, dtype=np.float32)
    return acc


# revision 15
# speedup vs baseline: 1.0411x; 1.0411x over previous
"""
BasicCrossAttention Trainium2 kernel (8 NeuronCores, SPMD head-parallel).

Sharding: 16 heads split across 8 cores (2 heads/core).  Each core computes
Q/K/V projections for its 2 heads (column-sharded Wq/Wk/Wv), per-head QK
LayerNorm, full cross-attention over B*H_local, and a row-sharded partial of
the output projection.  The host sums the 8 partial outputs and adds bias.

Device math is bf16 matmuls with fp32 PSUM accumulation.

v1 changes over the original baseline (396us):
  - x1/x2 are pre-transposed on the HOST -> xT tiles load as plain strided
    DMAs on the hw DGE (no serialized Sync-queue DMA-xbar transposes, no
    256B descriptor shatter).
  - softmax normalizer reciprocal uses the single-instruction
    reciprocal_approx_fast DVE op (~640ns vs 3.2us bit-exact iterative
    divide) -- removes the 16 head-of-line Vector-queue stalls that idled
    the whole machine ~4-5us each.
  - output partials are written bf16 (host sums in f32).
  - emission schedule weaves projection row-groups, attention chunks, and
    output-projection chunks so the ACT engine (softmax exp, the scarcest
    resource after PE) starts ~15us in and never drains until the tail.
"""

import os
import sys

for _p in ("/root/.axon_site", "/root/.axon_site/_ro/trn_rl_repo",
           "/root/.axon_site/_ro/pypackages", "/opt/trn_rl_repo"):
    if os.path.isdir(_p) and _p not in sys.path:
        sys.path.append(_p)

import numpy as np
import ml_dtypes
from contextlib import ExitStack

B = 2
N = 2048          # query rows (x1)
M = 2048          # key rows (x2)
DM = 1024         # d_model
H = 16            # total heads
HD = 64           # head dim
NCORES = 8
HL = H // NCORES  # heads per core = 2
LOC = HL * HD     # local feature width = 128
SCALE = 8.0 / HD  # mup scale
EPS = 1e-5

_COMPILED = None          # cached Bass program
LAST_RESULT = None        # BassKernelResults of last run (for profiling)


def _emit(ctx, tc, aps):
    import concourse.bass as bass
    from concourse import mybir
    from concourse.masks import make_identity

    nc = tc.nc
    f32 = mybir.dt.float32
    bf16 = mybir.dt.bfloat16
    AF = mybir.ActivationFunctionType
    OP = mybir.AluOpType

    x1T, x2T, wqT, wkT, wvT, wp, ln_g, ln_b, out = (
        aps["x1T"], aps["x2T"], aps["wqT"], aps["wkT"], aps["wvT"],
        aps["wp"], aps["ln_g"], aps["ln_b"], aps["out"])

    const = ctx.enter_context(tc.tile_pool(name="const", bufs=1))
    xT_pool = ctx.enter_context(tc.tile_pool(name="xTp", bufs=2))
    nat_pool = ctx.enter_context(tc.tile_pool(name="natp", bufs=10))
    stat_pool = ctx.enter_context(tc.tile_pool(name="statp", bufs=3))
    big_pool = ctx.enter_context(tc.tile_pool(name="bigp", bufs=2))
    pT_pool = ctx.enter_context(tc.tile_pool(name="pTp", bufs=3))
    out_pool = ctx.enter_context(tc.tile_pool(name="outp", bufs=2))
    bc_pool = ctx.enter_context(tc.tile_pool(name="bcp", bufs=2))
    ps128 = ctx.enter_context(tc.tile_pool(name="ps128", bufs=2, space="PSUM"))
    psbig = ctx.enter_context(tc.tile_pool(name="psbig", bufs=2, space="PSUM"))

    # ---------------- constants / weights ----------------
    ident = const.tile([128, 128], bf16)
    make_identity(nc, ident)

    # wkv layout: [in 128, fc 8, k(128) | v(128)]
    wkv_sb = const.tile([128, 8, 2 * LOC], bf16)
    nc.gpsimd.dma_start(wkv_sb[:, :, 0:LOC],
                        wkT.rearrange("(a p) o -> p a o", p=128))
    nc.gpsimd.dma_start(wkv_sb[:, :, LOC:2 * LOC],
                        wvT.rearrange("(a p) o -> p a o", p=128))
    wq_sb = const.tile([128, 8, LOC], bf16)
    nc.gpsimd.dma_start(wq_sb, wqT.rearrange("(a p) o -> p a o", p=128))
    wp_sb = const.tile([128, DM], bf16)
    nc.gpsimd.dma_start(wp_sb, wp)

    # ln params replicated per local feature: partition p <- param[p % 64]
    g_col = const.tile([128, 1], f32)
    nc.gpsimd.dma_start(g_col, bass.AP(tensor=ln_g.tensor, offset=ln_g.offset,
                                       ap=[[0, HL], [1, HD]]))
    b_col = const.tile([128, 1], f32)
    nc.gpsimd.dma_start(b_col, bass.AP(tensor=ln_b.tensor, offset=ln_b.offset,
                                       ap=[[0, HL], [1, HD]]))
    gq_col = const.tile([128, 1], f32)
    nc.vector.tensor_scalar_mul(gq_col, g_col, SCALE)
    bq_col = const.tile([128, 1], f32)
    nc.vector.tensor_scalar_mul(bq_col, b_col, SCALE)

    # (K/Q weights arrive mean-centered per head from the host, so
    # projections are zero-mean per head; only E[x^2] is needed on-device.)

    # persistent per-batch tiles (bufs=2 -> both batches in flight)
    kT = [None, None]
    qT = [None, None]
    Vt = [None, None]
    hoT = [None, None]

    # ---------------- phase generators ----------------
    xts = {}

    def load_x(b, is_q, rg):
        """Issue the (pre-transposed) x row-group load; prefetched one
        projection phase ahead so the transfer hides under compute."""
        src = x1T if is_q else x2T
        xT = xT_pool.tile([128, 8, 1024], bf16, tag="xT",
                          name=f"xT{b}{int(is_q)}{rg}")
        nc.sync.dma_start(
            xT, src[b, :, rg * 1024:(rg + 1) * 1024].rearrange(
                "(a p) n -> p a n", p=128))
        xts[(b, is_q, rg)] = xT

    def proj(b, is_q, rg):
        """Project K|V (from x2T) or Q (from x1T) for one 1024-row group.

        Emits the xT load as one big strided DMA, then the 8 m-tile matmul
        groups + LN stats, then the batched rstd, then per m-tile the
        normalize + transpose + scale-out.  Yields once after each m-tile's
        kT/qT columns are fully written (so attention over that tile may be
        emitted by the driver)."""
        if is_q and rg == 0:
            qT[b] = big_pool.tile([128, N], bf16, tag="qT", name=f"qT{b}")
        if not is_q and rg == 0:
            kT[b] = big_pool.tile([128, M], bf16, tag="kT", name=f"kT{b}")
            Vt[b] = big_pool.tile([128, 16, 2 * (HD + 1)], bf16, tag="V",
                                  name=f"V{b}")
        w_sb = wq_sb if is_q else wkv_sb
        nout = LOC if is_q else 2 * LOC
        dst = qT[b] if is_q else kT[b]
        gc, bc = (gq_col, bq_col) if is_q else (g_col, b_col)
        xT = xts.pop((b, is_q, rg))

        s2g = stat_pool.tile([128, 8, HL], f32, tag="s2g",
                             name=f"s2g{b}{int(is_q)}{rg}")
        raws = []
        for mi in range(8):
            mt = rg * 8 + mi  # global 128-row tile index
            rs = slice(mi * 128, (mi + 1) * 128)
            ps = ps128.tile([128, nout], f32, tag="ps128",
                            name=f"ps{b}{int(is_q)}{mt}")
            for fc in range(8):
                nc.tensor.matmul(ps, lhsT=xT[:, fc, rs],
                                 rhs=w_sb[:, fc, :],
                                 start=(fc == 0), stop=(fc == 7))
            raw = nat_pool.tile([128, LOC], bf16, tag="raw", bufs=10,
                                name=f"raw{b}{int(is_q)}{mt}")
            nc.vector.tensor_copy(raw, ps[:, 0:LOC])
            raws.append(raw)
            # E[x^2] per head for LN (weights are centered)
            sq = nat_pool.tile([128, LOC], f32, tag="sq", bufs=2)
            nc.vector.tensor_mul(sq, raw, raw)
            nc.vector.reduce_sum(s2g[:, mi, :],
                                 sq.rearrange("p (h d) -> p h d", h=HL),
                                 axis=mybir.AxisListType.X)
            if not is_q:
                vt = Vt[b][:, mt, :]
                nc.gpsimd.memset(vt[:, HD::HD + 1], 1.0)
                vt3 = bass.AP(tensor=vt.tensor, offset=vt.offset,
                              ap=[vt.ap[0], [HD + 1, HL], [1, HD]])
                nc.vector.tensor_copy(
                    vt3, ps[:, LOC:2 * LOC].rearrange(
                        "p (h x) -> p h x", h=HL))
        # per-row-group rstd on DVE: rsqrt(var+eps) via linear seed
        # + 3 Newton steps (keeps ACT exclusively on softmax exp)
        rstdg = stat_pool.tile([128, 8, HL], f32, tag="rstdg")
        y = rstdg.rearrange("p a b -> p (a b)")
        var = stat_pool.tile([128, 8 * HL], f32, tag="lnvar")
        tnr = stat_pool.tile([128, 8 * HL], f32, tag="lntnr")
        nc.vector.tensor_scalar(var, s2g.rearrange("p a b -> p (a b)"),
                                1.0 / HD, EPS, op0=OP.mult, op1=OP.add)
        nc.vector.tensor_scalar(y, var, -0.315, 1.43,
                                op0=OP.mult, op1=OP.add)
        for _ in range(3):
            nc.vector.tensor_mul(tnr, y, y)
            nc.vector.tensor_mul(tnr, tnr, var)
            nc.vector.tensor_scalar(tnr, tnr, -0.5, 1.5,
                                    op0=OP.mult, op1=OP.add)
            nc.vector.tensor_mul(y, y, tnr)
        for mi in range(8):
            mt = rg * 8 + mi
            nrm = nat_pool.tile([128, LOC], bf16, tag="nrm", bufs=3)
            for h in range(HL):
                hs = slice(h * HD, (h + 1) * HD)
                nc.vector.tensor_scalar(
                    nrm[:, hs], raws[mi][:, hs],
                    rstdg[:, mi, h:h + 1], None, op0=OP.mult)
            tps = ps128.tile([128, 128], bf16, tag="ps128",
                             name=f"tps{b}{int(is_q)}{mt}")
            nc.tensor.transpose(tps, nrm, ident)
            nc.vector.tensor_scalar(
                dst[:, mt * 128:(mt + 1) * 128], tps, gc, bc,
                op0=OP.mult, op1=OP.add)
            yield  # m-tile `mt` of dst is complete

    def attn_mc(b, nc4):
        """S^T -> exp -> (V|1)^T @ P^T for one 512-query chunk; one yield
        per key m-tile (16 total).  Must be pulled only after the
        corresponding kT/Vt m-tile has been emitted."""
        ns = slice(nc4 * 512, (nc4 + 1) * 512)
        av = psbig.tile([128, 1024], f32, tag="av", bufs=1,
                        name=f"av{b}{nc4}")
        avs[(b, nc4)] = av
        for mc in range(16):
            mcs = slice(mc * 128, (mc + 1) * 128)
            st = psbig.tile([128, 1024], f32, tag="st",
                            name=f"st{b}{nc4}{mc}")
            for h in range(HL):
                nc.tensor.matmul(st[:, h * 512:(h + 1) * 512],
                                 lhsT=kT[b][h * HD:(h + 1) * HD, mcs],
                                 rhs=qT[b][h * HD:(h + 1) * HD, ns],
                                 start=True, stop=True)
            pT = pT_pool.tile([128, 1024], bf16, tag="pT")
            nc.scalar.activation(pT, st, AF.Exp)
            for h in range(HL):
                nc.tensor.matmul(
                    av[0:HD + 1, h * 512:(h + 1) * 512],
                    lhsT=Vt[b][:, mc, h * (HD + 1):(h + 1) * (HD + 1)],
                    rhs=pT[:, h * 512:(h + 1) * 512],
                    start=(mc == 0), stop=(mc == 15),
                    skip_group_check=True)
            yield

    avs = {}

    def attn_norm(b, nc4):
        """Drain the accumulator, normalize by the softmax sum, write hoT."""
        if hoT[b] is None:
            hoT[b] = big_pool.tile([128, N], bf16, tag="hoT", name=f"hoT{b}")
        ns = slice(nc4 * 512, (nc4 + 1) * 512)
        av = avs.pop((b, nc4))
        # drain the accumulator to SBUF fast (frees the PSUM bank so the
        # next chunk's AV matmuls never stall on the normalize chain)
        av_sb = bc_pool.tile([HD + 1, 1024], f32, tag="avsb")
        nc.any.tensor_copy(av_sb, av[0:HD + 1, :])
        # 1/Z via exp(-ln Z) on ACT: both functions share one table set
        # (natural_log_exp_and_others), and ACT row ops don't head-of-line
        # block the DVE the way the 3.2us iterative reciprocal did.
        lnz = bc_pool.tile([1, 1024], f32, tag="lnz")
        nc.scalar.activation(lnz, av_sb[HD:HD + 1, :], AF.Ln)
        recip = bc_pool.tile([1, 1024], f32, tag="recip")
        nc.scalar.activation(recip, lnz, AF.Exp, scale=-1.0)
        for h in range(HL):
            hs = slice(h * HD, (h + 1) * HD)
            sl = slice(h * 512, (h + 1) * 512)
            bcast = bc_pool.tile([HD, 512], f32, tag="bcast")
            nc.gpsimd.partition_broadcast(bcast, recip[:, sl])
            nc.gpsimd.tensor_mul(hoT[b][hs, ns], av_sb[0:HD, sl], bcast)

    def outp(b, nc4):
        """Output projection partials for the 4 query m-tiles of chunk nc4.
        Must be emitted after attn_norm(b, nc4)."""
        for nt in range(4 * nc4, 4 * nc4 + 4):
            osb = out_pool.tile([128, DM], bf16, tag="osb")
            for oc in range(4):
                fps = ps128.tile([128, 256], f32, tag="ps128",
                                 name=f"fps{b}{nt}{oc}")
                nc.tensor.matmul(fps,
                                 lhsT=hoT[b][:, nt * 128:(nt + 1) * 128],
                                 rhs=wp_sb[:, oc * 256:(oc + 1) * 256],
                                 start=True, stop=True)
                nc.any.tensor_copy(osb[:, oc * 256:(oc + 1) * 256], fps)
                if oc == 3:
                    nc.sync.dma_start(
                        out[b, nt * 128:(nt + 1) * 128, :], osb)
                yield

    # ---------------- drivers ----------------
    def run(g):
        for _ in g:
            pass

    def weave(producer, *consumers):
        """Drain `producer`; after each of its yields (one per completed
        m-tile), pull `n` steps from each (gen, n) consumer."""
        for _ in producer:
            for g, n in consumers:
                for _ in range(n):
                    next(g, None)

    A = {}
    O = {}

    # projection phase order; each phase's x load is issued one phase ahead
    SEQ = [(0, True, 0), (0, False, 0), (0, False, 1), (0, True, 1),
           (1, False, 0), (1, False, 1), (1, True, 0), (1, True, 1)]
    load_x(*SEQ[0])
    load_x(*SEQ[1])

    def wproj(k, *consumers):
        if k + 2 < len(SEQ):
            load_x(*SEQ[k + 2])
        weave(proj(*SEQ[k]), *consumers)

    # --- batch 0 production, woven with batch-0 attention ---
    wproj(0)                                           # Q(b0) tiles 0-7
    A[0, 0] = attn_mc(0, 0)
    wproj(1, (A[0, 0], 1))                             # K|V rg0 ; attn mc 0-7
    wproj(2, (A[0, 0], 1))                             # K|V rg1 ; attn mc 8-15
    attn_norm(0, 0)
    A[0, 1] = attn_mc(0, 1)
    wproj(3, (A[0, 1], 2))                             # Q rg1 ; attn chunk 1
    attn_norm(0, 1)
    # --- batch 1 production, woven with attn(0) tail + outp(0) ---
    A[0, 2] = attn_mc(0, 2)
    O[0, 0] = outp(0, 0)
    O[0, 1] = outp(0, 1)
    wproj(4, (A[0, 2], 1), (O[0, 0], 2))
    wproj(5, (A[0, 2], 1), (O[0, 1], 2))
    attn_norm(0, 2)
    A[0, 3] = attn_mc(0, 3)
    O[0, 2] = outp(0, 2)
    wproj(6, (A[0, 3], 2), (O[0, 2], 2))
    attn_norm(0, 3)
    A[1, 0] = attn_mc(1, 0)
    O[0, 3] = outp(0, 3)
    wproj(7, (A[1, 0], 2), (O[0, 3], 2))
    attn_norm(1, 0)
    # --- batch 1 attention + output drain ---
    for c in range(1, 4):
        A[1, c] = attn_mc(1, c)
        O[1, c - 1] = outp(1, c - 1)
        weave(A[1, c], (O[1, c - 1], 1))
        attn_norm(1, c)
    run(outp(1, 3))
    # safety: drain any generator with leftover steps (counts should be exact)
    for g in list(A.values()) + list(O.values()):
        run(g)


def _force_shared_exp_ln_table(arch):
    """Mutate the cached activation-table map so Exp and Ln both resolve to
    one table set (e.g. natural_log_exp_and_others).  Otherwise the compiler
    assigns them different sets and every softmax-normalizer Ln triggers a
    ~2.7us ACT table reload right next to the softmax Exp stream."""
    from concourse import hw_specs, mybir

    ex, ln = (mybir.ActivationFunctionType.Exp, mybir.ActivationFunctionType.Ln)
    tables = hw_specs.get_activation_tables(arch)  # functools.cached dict
    shared = [n for n, fns in tables.items() if ex in fns and ln in fns]
    if not shared:
        return
    for name, fns in tables.items():
        if name != shared[0]:
            fns.discard(ex)
            fns.discard(ln)


def _build():
    global _COMPILED
    if _COMPILED is not None:
        return _COMPILED
    import concourse.tile as tile
    from concourse import bacc, mybir

    nc = bacc.Bacc("TRN2", target_bir_lowering=False, debug=False,
                   enable_asserts=False)
    _force_shared_exp_ln_table(nc.m.arch)
    bf16 = mybir.dt.bfloat16
    f32 = mybir.dt.float32
    aps = {
        "x1T": nc.dram_tensor("x1T", [B, DM, N], bf16, kind="ExternalInput").ap(),
        "x2T": nc.dram_tensor("x2T", [B, DM, M], bf16, kind="ExternalInput").ap(),
        "wqT": nc.dram_tensor("wqT", [DM, LOC], bf16, kind="ExternalInput").ap(),
        "wkT": nc.dram_tensor("wkT", [DM, LOC], bf16, kind="ExternalInput").ap(),
        "wvT": nc.dram_tensor("wvT", [DM, LOC], bf16, kind="ExternalInput").ap(),
        "wp": nc.dram_tensor("wp", [LOC, DM], bf16, kind="ExternalInput").ap(),
        "ln_g": nc.dram_tensor("ln_g", [HD], f32, kind="ExternalInput").ap(),
        "ln_b": nc.dram_tensor("ln_b", [HD], f32, kind="ExternalInput").ap(),
        "out": nc.dram_tensor("out", [B, N, DM], bf16, kind="ExternalOutput").ap(),
    }
    with tile.TileContext(nc) as tc, ExitStack() as ctx:
        _emit(ctx, tc, aps)
    nc.compile()
    _COMPILED = nc
    return nc


def kernel(x1, x2, Wq, Wk, Wv, Wp, bp, ln_g, ln_b):
    global LAST_RESULT
    from concourse.bass_utils import run_bass_kernel_spmd

    nc = _build()
    bf = ml_dtypes.bfloat16
    x1T = np.ascontiguousarray(
        np.asarray(x1, dtype=np.float32).transpose(0, 2, 1)).astype(bf)
    x2T = np.ascontiguousarray(
        np.asarray(x2, dtype=np.float32).transpose(0, 2, 1)).astype(bf)
    Wq = np.asarray(Wq, dtype=np.float32)
    Wk = np.asarray(Wk, dtype=np.float32)
    Wv = np.asarray(Wv, dtype=np.float32)
    Wp = np.asarray(Wp, dtype=np.float32)
    # Mean-center K/Q weight rows per 64-row head block (LN mean folded into
    # the weights; the device then only needs E[x^2] per head per token).
    Wqh = Wq.reshape(H, HD, DM)
    Wq = (Wqh - Wqh.mean(axis=1, keepdims=True)).reshape(DM, DM)
    Wkh = Wk.reshape(H, HD, DM)
    Wk = (Wkh - Wkh.mean(axis=1, keepdims=True)).reshape(DM, DM)
    ln_g32 = np.ascontiguousarray(np.asarray(ln_g, dtype=np.float32))
    ln_b32 = np.ascontiguousarray(np.asarray(ln_b, dtype=np.float32))

    in_maps = []
    for c in range(NCORES):
        hs = slice(c * LOC, (c + 1) * LOC)
        in_maps.append({
            "x1T": x1T,
            "x2T": x2T,
            "wqT": np.ascontiguousarray(Wq[hs, :].T).astype(bf),
            "wkT": np.ascontiguousarray(Wk[hs, :].T).astype(bf),
            "wvT": np.ascontiguousarray(Wv[hs, :].T).astype(bf),
            "wp": np.ascontiguousarray(Wp[:, hs].T).astype(bf),
            "ln_g": ln_g32,
            "ln_b": ln_b32,
        })

    res = run_bass_kernel_spmd(nc, in_maps, core_ids=list(range(NCORES)))
    LAST_RESULT = res
    acc = np.zeros((B, N, DM), dtype=np.float32)
    for r in res.results:
        acc += np.asarray(r["out"], dtype=np.float32)
    acc += np.asarray(bp, dtype=np.float32)
    return acc


# revision 20
# speedup vs baseline: 1.3012x; 1.2498x over previous
"""
BasicCrossAttention Trainium2 kernel (8 NeuronCores, SPMD head-parallel).

Sharding: 16 heads split across 8 cores (2 heads/core).  Each core computes
Q/K/V projections for its 2 heads (column-sharded Wq/Wk/Wv), per-head QK
LayerNorm, full cross-attention over B*H_local, and a row-sharded partial of
the output projection.  The host sums the 8 partial outputs and adds bias.

Device math is bf16 matmuls with fp32 PSUM accumulation.

v1 changes over the original baseline (396us):
  - x1/x2 are pre-transposed on the HOST -> xT tiles load as plain strided
    DMAs on the hw DGE (no serialized Sync-queue DMA-xbar transposes, no
    256B descriptor shatter).
  - softmax normalizer reciprocal uses the single-instruction
    reciprocal_approx_fast DVE op (~640ns vs 3.2us bit-exact iterative
    divide) -- removes the 16 head-of-line Vector-queue stalls that idled
    the whole machine ~4-5us each.
  - output partials are written bf16 (host sums in f32).
  - emission schedule weaves projection row-groups, attention chunks, and
    output-projection chunks so the ACT engine (softmax exp, the scarcest
    resource after PE) starts ~15us in and never drains until the tail.
"""

import os
import sys

for _p in ("/root/.axon_site", "/root/.axon_site/_ro/trn_rl_repo",
           "/root/.axon_site/_ro/pypackages", "/opt/trn_rl_repo"):
    if os.path.isdir(_p) and _p not in sys.path:
        sys.path.append(_p)

import numpy as np
import ml_dtypes
from contextlib import ExitStack

B = 2
N = 2048          # query rows (x1)
M = 2048          # key rows (x2)
DM = 1024         # d_model
H = 16            # total heads
HD = 64           # head dim
NCORES = 8
HL = H // NCORES  # heads per core = 2
LOC = HL * HD     # local feature width = 128
SCALE = 8.0 / HD  # mup scale
EPS = 1e-5

_COMPILED = None          # cached Bass program
LAST_RESULT = None        # BassKernelResults of last run (for profiling)


def _emit(ctx, tc, aps):
    import concourse.bass as bass
    from concourse import mybir
    from concourse.masks import make_identity

    nc = tc.nc
    f32 = mybir.dt.float32
    bf16 = mybir.dt.bfloat16
    AF = mybir.ActivationFunctionType
    OP = mybir.AluOpType

    x1T, x2T, wqT, wkT, wvT, wp, ln_g, ln_b, out = (
        aps["x1T"], aps["x2T"], aps["wqT"], aps["wkT"], aps["wvT"],
        aps["wp"], aps["ln_g"], aps["ln_b"], aps["out"])

    const = ctx.enter_context(tc.tile_pool(name="const", bufs=1))
    xT_pool = ctx.enter_context(tc.tile_pool(name="xTp", bufs=2))
    nat_pool = ctx.enter_context(tc.tile_pool(name="natp", bufs=10))
    stat_pool = ctx.enter_context(tc.tile_pool(name="statp", bufs=3))
    big_pool = ctx.enter_context(tc.tile_pool(name="bigp", bufs=2))
    pT_pool = ctx.enter_context(tc.tile_pool(name="pTp", bufs=3))
    out_pool = ctx.enter_context(tc.tile_pool(name="outp", bufs=2))
    bc_pool = ctx.enter_context(tc.tile_pool(name="bcp", bufs=2))
    ps128 = ctx.enter_context(tc.tile_pool(name="ps128", bufs=2, space="PSUM"))
    psbig = ctx.enter_context(tc.tile_pool(name="psbig", bufs=2, space="PSUM"))

    # ---------------- constants / weights ----------------
    ident = const.tile([128, 128], bf16)
    make_identity(nc, ident)

    # wkv layout: [in 128, fc 8, k(128) | v(128)]
    wkv_sb = const.tile([128, 8, 2 * LOC], bf16)
    nc.sync.dma_start(wkv_sb[:, :, 0:LOC],
                        wkT.rearrange("(a p) o -> p a o", p=128))
    nc.sync.dma_start(wkv_sb[:, :, LOC:2 * LOC],
                        wvT.rearrange("(a p) o -> p a o", p=128))
    wq_sb = const.tile([128, 8, LOC], bf16)
    nc.sync.dma_start(wq_sb, wqT.rearrange("(a p) o -> p a o", p=128))
    wp_sb = const.tile([128, DM], bf16)
    nc.sync.dma_start(wp_sb, wp)

    # ln params replicated per local feature: partition p <- param[p % 64]
    g_col = const.tile([128, 1], f32)
    nc.sync.dma_start(g_col, bass.AP(tensor=ln_g.tensor, offset=ln_g.offset,
                                       ap=[[0, HL], [1, HD]]))
    b_col = const.tile([128, 1], f32)
    nc.sync.dma_start(b_col, bass.AP(tensor=ln_b.tensor, offset=ln_b.offset,
                                       ap=[[0, HL], [1, HD]]))
    gq_col = const.tile([128, 1], f32)
    nc.vector.tensor_scalar_mul(gq_col, g_col, SCALE)
    bq_col = const.tile([128, 1], f32)
    nc.vector.tensor_scalar_mul(bq_col, b_col, SCALE)
    ones_col = const.tile([1, HD], bf16)
    nc.vector.memset(ones_col, 1.0)

    # (K/Q weights arrive mean-centered per head from the host, so
    # projections are zero-mean per head; only E[x^2] is needed on-device.)

    # persistent per-batch tiles (bufs=2 -> both batches in flight)
    kT = [None, None]
    qT = [None, None]
    Vt = [None, None]
    hoT = [None, None]

    # ---------------- phase generators ----------------
    xts = {}

    def load_x(b, is_q, rg):
        """Issue the (pre-transposed) x row-group load; prefetched one
        projection phase ahead so the transfer hides under compute."""
        src = x1T if is_q else x2T
        xT = xT_pool.tile([128, 8, 1024], bf16, tag="xT",
                          name=f"xT{b}{int(is_q)}{rg}")
        nc.sync.dma_start(
            xT, src[b, :, rg * 1024:(rg + 1) * 1024].rearrange(
                "(a p) n -> p a n", p=128))
        xts[(b, is_q, rg)] = xT

    def proj(b, is_q, rg):
        """Project K|V (from x2T) or Q (from x1T) for one 1024-row group.

        Emits the xT load as one big strided DMA, then the 8 m-tile matmul
        groups + LN stats, then the batched rstd, then per m-tile the
        normalize + transpose + scale-out.  Yields once after each m-tile's
        kT/qT columns are fully written (so attention over that tile may be
        emitted by the driver)."""
        if is_q and rg == 0:
            qT[b] = big_pool.tile([128, N], bf16, tag="qT", name=f"qT{b}")
        if not is_q and rg == 0:
            kT[b] = big_pool.tile([128, M], bf16, tag="kT", name=f"kT{b}")
            Vt[b] = big_pool.tile([128, 16, 2 * (HD + 1)], bf16, tag="V",
                                  name=f"V{b}")
        w_sb = wq_sb if is_q else wkv_sb
        nout = LOC if is_q else 2 * LOC
        dst = qT[b] if is_q else kT[b]
        gc, bc = (gq_col, bq_col) if is_q else (g_col, b_col)
        xT = xts.pop((b, is_q, rg))

        s2g = stat_pool.tile([128, 8, HL], f32, tag="s2g",
                             name=f"s2g{b}{int(is_q)}{rg}")
        raws = []
        for mi in range(8):
            mt = rg * 8 + mi  # global 128-row tile index
            rs = slice(mi * 128, (mi + 1) * 128)
            ps = ps128.tile([128, nout], f32, tag="ps128",
                            name=f"ps{b}{int(is_q)}{mt}")
            for fc in range(8):
                nc.tensor.matmul(ps, lhsT=xT[:, fc, rs],
                                 rhs=w_sb[:, fc, :],
                                 start=(fc == 0), stop=(fc == 7))
            raw = nat_pool.tile([128, LOC], bf16, tag="raw", bufs=10,
                                name=f"raw{b}{int(is_q)}{mt}")
            nc.vector.tensor_copy(raw, ps[:, 0:LOC])
            raws.append(raw)
            # E[x^2] per head for LN (weights are centered)
            sq = nat_pool.tile([128, LOC], f32, tag="sq", bufs=2)
            nc.vector.tensor_mul(sq, raw, raw)
            nc.vector.reduce_sum(s2g[:, mi, :],
                                 sq.rearrange("p (h d) -> p h d", h=HL),
                                 axis=mybir.AxisListType.X)
            if not is_q:
                vt = Vt[b][:, mt, :]
                nc.vector.memset(vt[:, HD::HD + 1], 1.0)
                vt3 = bass.AP(tensor=vt.tensor, offset=vt.offset,
                              ap=[vt.ap[0], [HD + 1, HL], [1, HD]])
                nc.vector.tensor_copy(
                    vt3, ps[:, LOC:2 * LOC].rearrange(
                        "p (h x) -> p h x", h=HL))
        # per-row-group rstd on DVE: rsqrt(var+eps) via linear seed
        # + 3 Newton steps (keeps ACT exclusively on softmax exp)
        rstdg = stat_pool.tile([128, 8, HL], f32, tag="rstdg")
        y = rstdg.rearrange("p a b -> p (a b)")
        var = stat_pool.tile([128, 8 * HL], f32, tag="lnvar")
        tnr = stat_pool.tile([128, 8 * HL], f32, tag="lntnr")
        nc.vector.tensor_scalar(var, s2g.rearrange("p a b -> p (a b)"),
                                1.0 / HD, EPS, op0=OP.mult, op1=OP.add)
        nc.vector.tensor_scalar(y, var, -0.315, 1.43,
                                op0=OP.mult, op1=OP.add)
        for _ in range(3):
            nc.vector.tensor_mul(tnr, y, y)
            nc.vector.tensor_mul(tnr, tnr, var)
            nc.vector.tensor_scalar(tnr, tnr, -0.5, 1.5,
                                    op0=OP.mult, op1=OP.add)
            nc.vector.tensor_mul(y, y, tnr)
        for mi in range(8):
            mt = rg * 8 + mi
            nrm = nat_pool.tile([128, LOC], bf16, tag="nrm", bufs=3)
            for h in range(HL):
                hs = slice(h * HD, (h + 1) * HD)
                nc.vector.tensor_scalar(
                    nrm[:, hs], raws[mi][:, hs],
                    rstdg[:, mi, h:h + 1], None, op0=OP.mult)
            tps = ps128.tile([128, 128], bf16, tag="ps128",
                             name=f"tps{b}{int(is_q)}{mt}")
            nc.tensor.transpose(tps, nrm, ident)
            nc.vector.tensor_scalar(
                dst[:, mt * 128:(mt + 1) * 128], tps, gc, bc,
                op0=OP.mult, op1=OP.add)
            yield  # m-tile `mt` of dst is complete

    def attn_mc(b, nc4):
        """S^T -> exp -> (V|1)^T @ P^T for one 512-query chunk; one yield
        per key m-tile (16 total).  Must be pulled only after the
        corresponding kT/Vt m-tile has been emitted."""
        ns = slice(nc4 * 512, (nc4 + 1) * 512)
        av = psbig.tile([128, 1024], f32, tag="av", bufs=1,
                        name=f"av{b}{nc4}")
        avs[(b, nc4)] = av
        for mc in range(16):
            mcs = slice(mc * 128, (mc + 1) * 128)
            st = psbig.tile([128, 1024], f32, tag="st",
                            name=f"st{b}{nc4}{mc}")
            for h in range(HL):
                nc.tensor.matmul(st[:, h * 512:(h + 1) * 512],
                                 lhsT=kT[b][h * HD:(h + 1) * HD, mcs],
                                 rhs=qT[b][h * HD:(h + 1) * HD, ns],
                                 start=True, stop=True)
            pT = pT_pool.tile([128, 1024], bf16, tag="pT")
            nc.scalar.activation(pT, st, AF.Exp)
            for h in range(HL):
                nc.tensor.matmul(
                    av[0:HD + 1, h * 512:(h + 1) * 512],
                    lhsT=Vt[b][:, mc, h * (HD + 1):(h + 1) * (HD + 1)],
                    rhs=pT[:, h * 512:(h + 1) * 512],
                    start=(mc == 0), stop=(mc == 15),
                    skip_group_check=True)
            yield

    avs = {}

    def attn_norm(b, nc4):
        """Drain the accumulator, normalize by the softmax sum, write hoT."""
        if hoT[b] is None:
            hoT[b] = big_pool.tile([128, N], bf16, tag="hoT", name=f"hoT{b}")
        ns = slice(nc4 * 512, (nc4 + 1) * 512)
        av = avs.pop((b, nc4))
        # drain the accumulator to SBUF fast (frees the PSUM bank so the
        # next chunk's AV matmuls never stall on the normalize chain)
        av_sb = bc_pool.tile([HD + 1, 1024], f32, tag="avsb")
        nc.any.tensor_copy(av_sb, av[0:HD + 1, :])
        # 1/Z via exp(-ln Z) on ACT: both functions share one table set
        # (natural_log_exp_and_others), and ACT row ops don't head-of-line
        # block the DVE the way the 3.2us iterative reciprocal did.
        lnz = bc_pool.tile([1, 1024], f32, tag="lnz")
        nc.scalar.activation(lnz, av_sb[HD:HD + 1, :], AF.Ln)
        recip = bc_pool.tile([1, 1024], bf16, tag="recip")
        nc.scalar.activation(recip, lnz, AF.Exp, scale=-1.0)
        # Broadcast 1/Z across the 64 feature partitions with a K=1 matmul
        # (ones[1,64].T @ recip[1,1024]) into the just-freed av PSUM slot.
        # (gpsimd partition_broadcast costs a ~5us library swap per op-type
        # switch, which serialized the whole back half of the kernel.)
        bc_ps = psbig.tile([HD, 1024], f32, tag="av", bufs=1,
                           name=f"bc{b}{nc4}")
        for h in range(HL):  # one matmul per PSUM bank (N=512 each)
            sl = slice(h * 512, (h + 1) * 512)
            nc.tensor.matmul(bc_ps[:, sl], lhsT=ones_col, rhs=recip[:, sl],
                             start=True, stop=True)
        for h in range(HL):
            hs = slice(h * HD, (h + 1) * HD)
            sl = slice(h * 512, (h + 1) * 512)
            nc.vector.tensor_mul(hoT[b][hs, ns], av_sb[0:HD, sl],
                                 bc_ps[0:HD, sl])

    def outp(b, nc4):
        """Output projection partials for the 4 query m-tiles of chunk nc4.
        Must be emitted after attn_norm(b, nc4)."""
        for nt in range(4 * nc4, 4 * nc4 + 4):
            osb = out_pool.tile([128, DM], bf16, tag="osb")
            for oc in range(4):
                fps = ps128.tile([128, 256], f32, tag="ps128",
                                 name=f"fps{b}{nt}{oc}")
                nc.tensor.matmul(fps,
                                 lhsT=hoT[b][:, nt * 128:(nt + 1) * 128],
                                 rhs=wp_sb[:, oc * 256:(oc + 1) * 256],
                                 start=True, stop=True)
                nc.any.tensor_copy(osb[:, oc * 256:(oc + 1) * 256], fps)
                if oc == 3:
                    nc.sync.dma_start(
                        out[b, nt * 128:(nt + 1) * 128, :], osb)
                yield

    # ---------------- drivers ----------------
    def run(g):
        for _ in g:
            pass

    def weave(producer, *consumers):
        """Drain `producer`; after each of its yields (one per completed
        m-tile), pull `n` steps from each (gen, n) consumer."""
        for _ in producer:
            for g, n in consumers:
                for _ in range(n):
                    next(g, None)

    A = {}
    O = {}

    # projection phase order; each phase's x load is issued one phase ahead
    SEQ = [(0, True, 0), (0, False, 0), (0, False, 1), (0, True, 1),
           (1, False, 0), (1, False, 1), (1, True, 0), (1, True, 1)]
    load_x(*SEQ[0])
    load_x(*SEQ[1])

    def wproj(k, *consumers):
        if k + 2 < len(SEQ):
            load_x(*SEQ[k + 2])
        weave(proj(*SEQ[k]), *consumers)

    # --- batch 0 production, woven with batch-0 attention ---
    wproj(0)                                           # Q(b0) tiles 0-7
    A[0, 0] = attn_mc(0, 0)
    wproj(1, (A[0, 0], 1))                             # K|V rg0 ; attn mc 0-7
    wproj(2, (A[0, 0], 1))                             # K|V rg1 ; attn mc 8-15
    attn_norm(0, 0)
    A[0, 1] = attn_mc(0, 1)
    wproj(3, (A[0, 1], 2))                             # Q rg1 ; attn chunk 1
    attn_norm(0, 1)
    # --- batch 1 production, woven with attn(0) tail + outp(0) ---
    A[0, 2] = attn_mc(0, 2)
    O[0, 0] = outp(0, 0)
    O[0, 1] = outp(0, 1)
    wproj(4, (A[0, 2], 1), (O[0, 0], 2))
    wproj(5, (A[0, 2], 1), (O[0, 1], 2))
    attn_norm(0, 2)
    A[0, 3] = attn_mc(0, 3)
    O[0, 2] = outp(0, 2)
    wproj(6, (A[0, 3], 2), (O[0, 2], 2))
    attn_norm(0, 3)
    A[1, 0] = attn_mc(1, 0)
    O[0, 3] = outp(0, 3)
    wproj(7, (A[1, 0], 2), (O[0, 3], 2))
    attn_norm(1, 0)
    # --- batch 1 attention + output drain ---
    for c in range(1, 4):
        A[1, c] = attn_mc(1, c)
        O[1, c - 1] = outp(1, c - 1)
        weave(A[1, c], (O[1, c - 1], 1))
        attn_norm(1, c)
    run(outp(1, 3))
    # safety: drain any generator with leftover steps (counts should be exact)
    for g in list(A.values()) + list(O.values()):
        run(g)


def _force_shared_exp_ln_table(arch):
    """Mutate the cached activation-table map so Exp and Ln both resolve to
    one table set (e.g. natural_log_exp_and_others).  Otherwise the compiler
    assigns them different sets and every softmax-normalizer Ln triggers a
    ~2.7us ACT table reload right next to the softmax Exp stream."""
    from concourse import hw_specs, mybir

    ex, ln = (mybir.ActivationFunctionType.Exp, mybir.ActivationFunctionType.Ln)
    tables = hw_specs.get_activation_tables(arch)  # functools.cached dict
    shared = [n for n, fns in tables.items() if ex in fns and ln in fns]
    if not shared:
        return
    for name, fns in tables.items():
        if name != shared[0]:
            fns.discard(ex)
            fns.discard(ln)


def _build():
    global _COMPILED
    if _COMPILED is not None:
        return _COMPILED
    import concourse.tile as tile
    from concourse import bacc, mybir

    nc = bacc.Bacc("TRN2", target_bir_lowering=False, debug=False,
                   enable_asserts=False)
    _force_shared_exp_ln_table(nc.m.arch)
    bf16 = mybir.dt.bfloat16
    f32 = mybir.dt.float32
    aps = {
        "x1T": nc.dram_tensor("x1T", [B, DM, N], bf16, kind="ExternalInput").ap(),
        "x2T": nc.dram_tensor("x2T", [B, DM, M], bf16, kind="ExternalInput").ap(),
        "wqT": nc.dram_tensor("wqT", [DM, LOC], bf16, kind="ExternalInput").ap(),
        "wkT": nc.dram_tensor("wkT", [DM, LOC], bf16, kind="ExternalInput").ap(),
        "wvT": nc.dram_tensor("wvT", [DM, LOC], bf16, kind="ExternalInput").ap(),
        "wp": nc.dram_tensor("wp", [LOC, DM], bf16, kind="ExternalInput").ap(),
        "ln_g": nc.dram_tensor("ln_g", [HD], f32, kind="ExternalInput").ap(),
        "ln_b": nc.dram_tensor("ln_b", [HD], f32, kind="ExternalInput").ap(),
        "out": nc.dram_tensor("out", [B, N, DM], bf16, kind="ExternalOutput").ap(),
    }
    with tile.TileContext(nc) as tc, ExitStack() as ctx:
        _emit(ctx, tc, aps)
    nc.compile()
    _COMPILED = nc
    return nc


def kernel(x1, x2, Wq, Wk, Wv, Wp, bp, ln_g, ln_b):
    global LAST_RESULT
    from concourse.bass_utils import run_bass_kernel_spmd

    nc = _build()
    bf = ml_dtypes.bfloat16
    x1T = np.ascontiguousarray(
        np.asarray(x1, dtype=np.float32).transpose(0, 2, 1)).astype(bf)
    x2T = np.ascontiguousarray(
        np.asarray(x2, dtype=np.float32).transpose(0, 2, 1)).astype(bf)
    Wq = np.asarray(Wq, dtype=np.float32)
    Wk = np.asarray(Wk, dtype=np.float32)
    Wv = np.asarray(Wv, dtype=np.float32)
    Wp = np.asarray(Wp, dtype=np.float32)
    # Mean-center K/Q weight rows per 64-row head block (LN mean folded into
    # the weights; the device then only needs E[x^2] per head per token).
    Wqh = Wq.reshape(H, HD, DM)
    Wq = (Wqh - Wqh.mean(axis=1, keepdims=True)).reshape(DM, DM)
    Wkh = Wk.reshape(H, HD, DM)
    Wk = (Wkh - Wkh.mean(axis=1, keepdims=True)).reshape(DM, DM)
    ln_g32 = np.ascontiguousarray(np.asarray(ln_g, dtype=np.float32))
    ln_b32 = np.ascontiguousarray(np.asarray(ln_b, dtype=np.float32))

    in_maps = []
    for c in range(NCORES):
        hs = slice(c * LOC, (c + 1) * LOC)
        in_maps.append({
            "x1T": x1T,
            "x2T": x2T,
            "wqT": np.ascontiguousarray(Wq[hs, :].T).astype(bf),
            "wkT": np.ascontiguousarray(Wk[hs, :].T).astype(bf),
            "wvT": np.ascontiguousarray(Wv[hs, :].T).astype(bf),
            "wp": np.ascontiguousarray(Wp[:, hs].T).astype(bf),
            "ln_g": ln_g32,
            "ln_b": ln_b32,
        })

    res = run_bass_kernel_spmd(nc, in_maps, core_ids=list(range(NCORES)))
    LAST_RESULT = res
    acc = np.zeros((B, N, DM), dtype=np.float32)
    for r in res.results:
        acc += np.asarray(r["out"], dtype=np.float32)
    acc += np.asarray(bp, dtype=np.float32)
    return acc
